# revision 5
# baseline (speedup 1.0000x reference)
"""Trainium2 Bass kernel v2: teacher-forced GRU decoder (B=512, T=32, H=2048, E=4096).

Sharding: pure data-parallel over batch across 8 NeuronCores (64 seqs/core).
Per-core dataflow (feature-on-partitions, "transposed" layouts):
  Phase A: gx^T = W_ih @ X^T for all timesteps, fp8e4(x16 weight scale) with
           DoubleRow (256-row contraction tiles) -> fp16 gx scratch in DRAM,
           stored as [M3, q, 128, 4*BL] blocks (q = 4-step column group).
  Phase B: 32-step scan, gates-on-partitions. lhsT = W_hh^T tiles (fp8 x16,
           SBUF-resident), rhs = h^T (fp8). fp32 master h in 2 PSUM banks;
           6 PSUM banks hold gh^T per step. fp8 h pair-buffer doubles as the
           hall scratch tile (DMA per 2 steps).
  Phase C: logits via fp8 DoubleRow (h8 pairs x W_out^T x16); b_out added
           during the fused DVE psum evacuation (pre-broadcast to 128
           partitions host-side); log_softmax along E; fp16 output.
"""

import os
import sys

for _p in ("/opt/trn_rl_repo", "/root/.axon_site/_ro/trn_rl_repo"):
    if os.path.isdir(_p) and _p not in sys.path:
        sys.path.append(_p)

import numpy as np

import concourse.bass as bass
import concourse.mybir as mybir
import concourse.tile as tile
from concourse import bacc

F8 = mybir.dt.float8e4
F16 = mybir.dt.float16
F32 = mybir.dt.float32
AF = mybir.ActivationFunctionType
OP = mybir.AluOpType
DR = mybir.MatmulPerfMode.DoubleRow

NCORES = 8
SW = 16.0          # fp8 weight pre-scale
ISW = 1.0 / SW

NP_F8 = mybir.dt.np(F8)


def build(BL=64, T=32, H=2048, E=4096, phases="abc"):
    """Build the single-core Bass program (SPMD across cores)."""
    G3 = 3 * H
    TB = T * BL
    KH = H // 128           # h contraction tiles (16)
    KHP = KH // 2           # h contraction pairs for DoubleRow (8)
    KE = E // 128           # e contraction tiles (32)
    KEP = KE // 2           # e contraction pairs (16)
    M3 = G3 // 128          # gate-row tiles (48)
    MH = KH // 2            # i-tiles per gate-math group (8)
    MC = TB // 128          # phase-C row tiles == step pairs (16)
    NE = E // 512           # phase-C 512-col slices (8)
    PA_N = min(512, TB)     # phase-A psum tile cols
    NT_A = TB // PA_N       # phase-A col chunks (4)
    QC = min(4 * BL, TB)    # gx q-chunk cols (4 steps)
    QN = TB // QC           # number of q chunks (8)
    SPQ = QC // BL          # steps per q chunk (4)
    QPP = PA_N // QC        # q chunks per phase-A psum tile (2)
    assert T % 2 == 0

    nc = bacc.Bacc(target_bir_lowering=False, trn_type="TRN2")

    x8 = nc.declare_dram_parameter("x8", [E, TB], F8, isOutput=False)
    wih8 = nc.declare_dram_parameter("wih8", [M3, 128, KE, 128], F8,
                                     isOutput=False)
    whh8 = nc.declare_dram_parameter("whh8", [H, G3], F8, isOutput=False)
    wout8 = nc.declare_dram_parameter("wout8", [H, E], F8, isOutput=False)
    h0T = nc.declare_dram_parameter("h0T", [H, BL], F16, isOutput=False)
    biasA = nc.declare_dram_parameter("biasA", [128, M3], F32, isOutput=False)
    bhhN = nc.declare_dram_parameter("bhhN", [128, KH], F32, isOutput=False)
    bo16 = nc.declare_dram_parameter("bo16", [128, E], F16, isOutput=False)
    out_d = nc.declare_dram_parameter("out", [TB, E], F16, isOutput=True)

    gx_d = nc.dram_tensor("gx_scratch", [M3, QN, 128, QC], F16)
    hall_d = nc.dram_tensor("hall_scratch", [MC, 128, KH * 2 * BL], F8)

    x8_p = x8[:].rearrange("(k p) n -> p k n", p=128)         # (128, KE, TB)
    whh8_p = whh8[:].rearrange("(k p) j -> p k j", p=128)     # (128, KH, G3)
    wout8_p = wout8[:].rearrange("(k p) e -> p k e", p=128)   # (128, KH, E)
    h0T_p = h0T[:].rearrange("(k p) b -> p k b", p=128)       # (128, KH, BL)

    with tile.TileContext(nc) as tc:
        # Resident W_hh pool; DMAs go on the ACT HWDGE ring so they overlap
        # phase A's sync-ring traffic (x8 / wih / gx).
        whh_pool = tc.alloc_tile_pool(name="whh_res", bufs=1)
        whh_sb = [whh_pool.tile([128, G3], F8, tag=f"whh{k}", name=f"whh{k}")
                  for k in range(KH)]

        # ================= Phase A: gx = W_ih @ X^T (fp8 DoubleRow) ========
        with tc.tile_pool(name="pa_x", bufs=1) as pa_x, \
             tc.tile_pool(name="pa_small", bufs=1) as pa_small, \
             tc.tile_pool(name="pa_w", bufs=3) as pa_w, \
             tc.tile_pool(name="pa_g", bufs=6) as pa_g, \
             tc.tile_pool(name="pa_ps", bufs=8, space="PSUM") as pa_ps:
            ba_sb = pa_small.tile([128, M3], F32)
            nc.sync.dma_start(out=ba_sb, in_=biasA[:])
            x_tiles = [pa_x.tile([128, 8, TB], F8, tag=f"x{g}", name=f"x{g}")
                       for g in range(KE // 8)]
            for g in range(KE // 8):
                nc.sync.dma_start(out=x_tiles[g],
                                  in_=x8_p[:, 8 * g:8 * (g + 1)])
            for k in range(KH):
                nc.scalar.dma_start(out=whh_sb[k], in_=whh8_p[:, k])
            for m in range(M3):
                wsl = pa_w.tile([128, KE, 128], F8, tag="wsl")
                nc.sync.dma_start(out=wsl, in_=wih8[m])
                ps = [pa_ps.tile([128, PA_N], F32, tag="pa_psum",
                                 name=f"ps_m{m}_{n}")
                      for n in range(NT_A)]
                for k in range(KEP):
                    xg = x_tiles[k // 4]
                    ko = 2 * k - 8 * (k // 4)
                    for n in range(NT_A):
                        nc.tensor.matmul(
                            ps[n],
                            wsl[:, 2 * k:2 * k + 2, :],
                            xg[:, ko:ko + 2, n * PA_N:(n + 1) * PA_N],
                            start=(k == 0),
                            stop=(k == KEP - 1),
                            perf_mode=DR,
                        )
                for n in range(NT_A):
                    g = pa_g.tile([128, PA_N], F16, tag="gstage")
                    nc.vector.tensor_scalar(
                        out=g, in0=ps[n], scalar1=ISW,
                        scalar2=ba_sb[:, m:m + 1], op0=OP.mult, op1=OP.add)
                    nc.sync.dma_start(
                        out=gx_d[m, n * QPP:(n + 1) * QPP].rearrange(
                            "q p c -> p q c"),
                        in_=g.rearrange("p (q c) -> p q c", c=QC),
                    )

        # ================= Phase B: GRU scan =================
        # PSUM banks: 6 for gh^T (3 gates x 2 halves), 2 for fp32 master h.
        with tc.tile_pool(name="sc_small", bufs=1) as sc_small, \
             tc.tile_pool(name="gxq", bufs=2) as gxq_pool, \
             tc.tile_pool(name="h8p", bufs=2) as h8_pool, \
             tc.tile_pool(name="gate", bufs=2) as gate_pool, \
             tc.tile_pool(name="hops", bufs=2) as hops_pool, \
             tc.tile_pool(name="h32ps", bufs=1, space="PSUM") as h32_ps, \
             tc.tile_pool(name="sc_ps", bufs=6, space="PSUM") as sc_ps:

            bn_sb = sc_small.tile([128, KH], F32)
            nc.sync.dma_start(out=bn_sb, in_=bhhN[:])

            h0_sb = sc_small.tile([128, KH, BL], F16)
            nc.sync.dma_start(out=h0_sb, in_=h0T_p)
            h0_8 = sc_small.tile([128, KH, BL], F8)
            nc.vector.tensor_copy(out=h0_8, in_=h0_sb)
            # fp32 master h: bank hf holds i-tiles hf*8..hf*8+7 as (m, b)
            h32 = []
            for hf in range(2):
                hb = h32_ps.tile([128, MH * BL], F32, tag=f"h32_{hf}",
                                 name=f"h32_{hf}")
                nc.vector.tensor_copy(
                    out=hb.rearrange("p (k b) -> p k b", b=BL),
                    in_=h0_sb[:, hf * MH:(hf + 1) * MH])
                h32.append(hb)

            h8_pairs = []  # pair tile: [128, KH, 2, BL]

            def h_rhs(t, k):
                """fp8 h^T tile k for input of step t (h after step t-1)."""
                if t == 0:
                    return h0_8[:, k, :]
                return h8_pairs[(t - 1) // 2][:, k, (t - 1) % 2, :]

            for t in range(T if "b" in phases else 0):
                q, tq = t // SPQ, t % SPQ
                if tq == 0:
                    gxq = gxq_pool.tile([128, M3, QC], F16, tag="gxq")
                    nc.sync.dma_start(
                        out=gxq, in_=gx_d[:, q].rearrange("m p c -> p m c"))
                if t % 2 == 0:
                    h8c = h8_pool.tile([128, KH, 2, BL], F8, tag="h8")
                    h8_pairs.append(h8c)

                for hf in range(2):
                    # gh^T for i-tiles hf*8..hf*8+7, gates r,z,n in 3 banks
                    ps_gate = [sc_ps.tile([128, MH * BL], F32, tag="sc_psum",
                                          name=f"ps_t{t}_{hf}_{g}")
                               for g in range(3)]
                    for kp in range(2):
                        for g in range(3):
                            ps = ps_gate[g]
                            for s in range(MH):
                                j = g * KH + hf * MH + s
                                for k in range(kp * MH, (kp + 1) * MH):
                                    nc.tensor.matmul(
                                        ps[:, s * BL:(s + 1) * BL],
                                        whh_sb[k][:, j * 128:(j + 1) * 128],
                                        h_rhs(t, k),
                                        start=(kp == 0 and s == 0
                                               and k == 0),
                                        stop=(kp == 1 and s == MH - 1
                                              and k == KH - 1),
                                        skip_group_check=True,
                                    )
                    ks = slice(hf * MH, (hf + 1) * MH)
                    gx_r = gxq[:, ks, t % SPQ * BL:(t % SPQ + 1) * BL]
                    gx_z = gxq[:, KH + hf * MH:KH + (hf + 1) * MH,
                               tq * BL:(tq + 1) * BL]
                    gx_n = gxq[:, 2 * KH + hf * MH:2 * KH + (hf + 1) * MH,
                               tq * BL:(tq + 1) * BL]
                    psr = ps_gate[0].rearrange("p (s b) -> p s b", b=BL)
                    psz = ps_gate[1].rearrange("p (s b) -> p s b", b=BL)
                    psn = ps_gate[2]
                    h32h = h32[hf].rearrange("p (k b) -> p k b", b=BL)
                    # r / z: pre = psum/SW + gx ; sigmoid on ACT
                    pre_r = gate_pool.tile([128, MH, BL], F16, tag="pre_r")
                    nc.vector.scalar_tensor_tensor(
                        out=pre_r, in0=psr, scalar=ISW, in1=gx_r,
                        op0=OP.mult, op1=OP.add)
                    r_h = gate_pool.tile([128, MH, BL], F16, tag="r_h")
                    nc.scalar.activation(out=r_h, in_=pre_r, func=AF.Sigmoid)
                    pre_z = gate_pool.tile([128, MH, BL], F16, tag="pre_z")
                    nc.vector.scalar_tensor_tensor(
                        out=pre_z, in0=psz, scalar=ISW, in1=gx_z,
                        op0=OP.mult, op1=OP.add)
                    z_h = gate_pool.tile([128, MH, BL], F16, tag="z_h")
                    nc.scalar.activation(out=z_h, in_=pre_z, func=AF.Sigmoid)
                    # n: tanh(gx_n + r * (gh_n + bhh_n))
                    #   psn' = (psn + SW*bhh) * r   (still x SW)
                    for s in range(MH):
                        kg = hf * MH + s
                        nc.vector.scalar_tensor_tensor(
                            out=psn[:, s * BL:(s + 1) * BL],
                            in0=psn[:, s * BL:(s + 1) * BL],
                            scalar=bn_sb[:, kg:kg + 1],
                            in1=r_h[:, s, :],
                            op0=OP.add,
                            op1=OP.mult,
                        )
                    pre_n = gate_pool.tile([128, MH, BL], F16, tag="pre_n")
                    nc.vector.scalar_tensor_tensor(
                        out=pre_n,
                        in0=psn.rearrange("p (s b) -> p s b", b=BL),
                        scalar=ISW, in1=gx_n, op0=OP.mult, op1=OP.add)
                    n_h = gate_pool.tile([128, MH, BL], F16, tag="n_h")
                    nc.scalar.activation(out=n_h, in_=pre_n, func=AF.Tanh)
                    # h' = n + z * (h - n)
                    t4 = hops_pool.tile([128, MH, BL], F16, tag="t4")
                    nc.vector.tensor_sub(t4, h32h, n_h)
                    nc.vector.tensor_mul(t4, z_h, t4)
                    nc.vector.tensor_add(h32h, n_h, t4)
                    nc.vector.tensor_copy(out=h8c[:, ks, t % 2, :], in_=h32h)

                if t % 2 == 1:
                    nc.scalar.dma_start(
                        out=hall_d[t // 2],
                        in_=h8c.rearrange("p k u b -> p (k u b)"))

        whh_pool.release()

        # ================= Phase C: logits + log_softmax (fp8 DR) ==========
        with tc.tile_pool(name="c_small", bufs=1) as c_small, \
             tc.tile_pool(name="wout_res", bufs=1) as wo_pool, \
             tc.tile_pool(name="hs_in", bufs=3) as hs_pool, \
             tc.tile_pool(name="logits", bufs=3) as lg_pool, \
             tc.tile_pool(name="expbuf", bufs=3) as ex_pool, \
             tc.tile_pool(name="o16", bufs=3) as o16_pool, \
             tc.tile_pool(name="stats", bufs=12) as st_pool, \
             tc.tile_pool(name="c_ps", bufs=6, space="PSUM") as c_ps:
            bo_sb = c_small.tile([128, E], F16)
            nc.sync.dma_start(out=bo_sb, in_=bo16[:])
            wo_sb = [wo_pool.tile([128, 2, E], F8, tag=f"wo{kp}",
                                  name=f"wo{kp}")
                     for kp in range(KHP if "c" in phases else 0)]
            for kp in range(KHP if "c" in phases else 0):
                nc.sync.dma_start(out=wo_sb[kp],
                                  in_=wout8_p[:, 2 * kp:2 * kp + 2])
            for mt in range(MC if "c" in phases else 0):
                hs8 = hs_pool.tile([128, KH, 2, BL], F8, tag="hs8")
                nc.scalar.dma_start(
                    out=hs8.rearrange("p k u b -> p (k u b)"),
                    in_=hall_d[mt])
                lg = lg_pool.tile([128, E], F16, tag="lg")
                for n in range(NE):
                    nsl = slice(n * 512, (n + 1) * 512)
                    ps = c_ps.tile([128, 512], F32, tag="c_psum")
                    for kp in range(KHP):
                        # lhsT: [128, 2(kpair), 128(t,b)]
                        lhsT = hs8[:, 2 * kp:2 * kp + 2].rearrange(
                            "p k u b -> p k (u b)")
                        nc.tensor.matmul(
                            ps,
                            lhsT,
                            wo_sb[kp][:, :, nsl],
                            start=(kp == 0),
                            stop=(kp == KHP - 1),
                            perf_mode=DR,
                        )
                    nc.vector.scalar_tensor_tensor(
                        out=lg[:, nsl], in0=ps, scalar=ISW,
                        in1=bo_sb[:, nsl], op0=OP.mult, op1=OP.add)
                negmax = st_pool.tile([128, 1], F32, tag="negmax")
                nc.vector.tensor_reduce(
                    out=negmax, in_=lg, axis=mybir.AxisListType.X,
                    op=OP.max, negate=True)
                eb = ex_pool.tile([128, E], F16, tag="eb")
                sumexp = st_pool.tile([128, 1], F32, tag="sumexp")
                nc.scalar.activation(
                    out=eb, in_=lg, func=AF.Exp,
                    bias=negmax, scale=1.0, accum_out=sumexp)
                lse = st_pool.tile([128, 1], F32, tag="lse")
                nc.scalar.activation(out=lse, in_=sumexp, func=AF.Ln)
                negoff = st_pool.tile([128, 1], F32, tag="negoff")
                nc.vector.tensor_sub(negoff, negmax, lse)
                o16 = o16_pool.tile([128, E], F16, tag="o16")
                nc.vector.tensor_scalar_add(o16, lg, negoff)
                nc.sync.dma_start(
                    out=out_d[mt * 128:(mt + 1) * 128, :], in_=o16)

    nc.finalize()
    return nc


def _host_prep(context_batch, target_encs, sos, W_ih, W_hh, b_ih, b_hh,
               W_out, b_out, BL, T, H, E):
    """Build per-core input maps (numpy layout transforms only)."""
    G3 = 3 * H
    M3 = G3 // 128
    KE = E // 128
    KH = H // 128
    B = context_batch.shape[0]
    ncores = B // BL

    wihT = np.ascontiguousarray(W_ih.T)                      # [E, G3] f32
    wih8 = np.ascontiguousarray(
        (wihT * SW).reshape(KE, 128, M3, 128).transpose(2, 1, 0, 3)
    ).astype(NP_F8)                                          # [M3,128,KE,128]
    whh8 = (np.ascontiguousarray(W_hh.T) * SW).astype(NP_F8)   # [H, G3]
    wout8 = (np.ascontiguousarray(W_out.T) * SW).astype(NP_F8)  # [H, E]
    biasA = b_ih.astype(np.float32).copy()
    biasA[:2 * H] += b_hh[:2 * H].astype(np.float32)
    biasA = np.ascontiguousarray(biasA.reshape(M3, 128).T)
    bhhN = np.ascontiguousarray(
        (SW * b_hh[2 * H:].astype(np.float32)).reshape(KH, 128).T)
    bo16 = np.ascontiguousarray(
        np.broadcast_to(b_out.astype(np.float16), (128, E)))

    in_maps = []
    for c in range(ncores):
        sl = slice(c * BL, (c + 1) * BL)
        xc = np.empty((BL, T, E), np.float32)
        xc[:, 0, :] = sos
        xc[:, 1:, :] = target_encs[sl, :T - 1, :]
        x8 = np.ascontiguousarray(
            xc.transpose(2, 1, 0).reshape(E, T * BL)).astype(NP_F8)
        h0T = np.ascontiguousarray(context_batch[sl].T).astype(np.float16)
        in_maps.append({
            "x8": x8, "wih8": wih8, "whh8": whh8, "wout8": wout8,
            "h0T": h0T, "biasA": biasA, "bhhN": bhhN, "bo16": bo16,
        })
    return in_maps


_CACHE = {}


def kernel(context_batch, target_encs, sos, W_ih, W_hh, b_ih, b_hh,
           W_out, b_out, trace=False):
    context_batch = np.asarray(context_batch, np.float32)
    target_encs = np.asarray(target_encs, np.float32)
    sos = np.asarray(sos, np.float32)
    W_ih = np.asarray(W_ih, np.float32)
    W_hh = np.asarray(W_hh, np.float32)
    b_ih = np.asarray(b_ih, np.float32)
    b_hh = np.asarray(b_hh, np.float32)
    W_out = np.asarray(W_out, np.float32)
    b_out = np.asarray(b_out, np.float32)
    B, T, E = target_encs.shape
    H = context_batch.shape[1]
    BL = B // NCORES

    if "nc" not in _CACHE:
        _CACHE["nc"] = build(BL=BL, T=T, H=H, E=E)
    nc = _CACHE["nc"]

    in_maps = _host_prep(context_batch, target_encs, sos, W_ih, W_hh,
                         b_ih, b_hh, W_out, b_out, BL, T, H, E)

    from concourse.bass_utils import run_bass_kernel_spmd
    res = run_bass_kernel_spmd(nc, in_maps, list(range(NCORES)), trace=trace)

    outs = []
    for c in range(NCORES):
        o = res.results[c]["out"]            # (T*BL, E) f16, row = t*BL + b
        outs.append(o.reshape(T, BL, E).transpose(1, 0, 2))
    full = np.concatenate(outs, axis=0).astype(np.float32)
    if trace:
        _CACHE["last_exec_time_ns"] = res.exec_time_ns
    return full
